# revision 49
# baseline (speedup 1.0000x reference)
"""CovaBlock Trainium2 kernel (nn_CovaBlock_8589934592087).

reference:
  support (16,5,256,21,21) -> per-class covariance cov (16,256,256)
  query (64,256,21,21) -> l2-normalize over C per location ->
  sim[b,k,l] = qn[b,:,l]^T cov[k] qn[b,:,l]  -> out (64, 16*441)

Distribution over 8 NeuronCores:
  stage 1: shard K (2 classes/core) -> each core computes the upper
           triangular blocks of its 2 gram matrices from HOST-CENTERED
           support in bf16 (the symmetric lower-left 128x128 block is
           reconstructed on the host as the transpose of the upper-right).
  stage 2: data-parallel over B (8 queries/core); host computes
           L = chol(cov) so sim = ||L^T q||^2, and the kernel runs
             phase A: M1 = L^T q (3 matmuls/pair: the upper-right block
                      of triangular L is zero)
             squares: M1^2 PSUM->SBUF bf16; merged 2-slab ACT square for
                      3 of 4 pairs, DVE copy + SBUF self-mult for the
                      4th (only ACT can math straight from PSUM and its
                      square time alone would tie the PE)
             fold:    sq[d] + sq[d+128] (DVE/GPSIMD bf16 add, 2x mode)
             phase B: partition-sum of the folded squares; rows
                      k < K-S2_RED via one ones-column matmul per (b,k)
                      accumulated into one PSUM bank per b by one-hot
                      column lhsT packing (no tile_position), the last
                      S2_RED rows via gpsimd.tensor_reduce(axis=C) so
                      the otherwise-idle GPSIMD engine relieves the PE
                      (the tensor engine is the overall bottleneck)
  The exact l2 normalization (and the 1/(N-1) covariance scale, folded
  into L) is applied as a final host-side scale 1/(||q||+1e-8)^2.

fp8 usage: the support tensor is fp8 e4m3 (the covariance only needs
~0.1% accuracy and this halves stage-1 DMA); stage 1's gram runs fp8
DoubleRow (2 row-tiles per matmul, verified stable + bit-deterministic
over 11 runs). Stage 2 stays all-bf16: a full-scale DoubleRow phase B
produced wrong results and intermittent NRT crashes on this HW (matching
a previous session's findings), so S2_DR stays False.
"""
import sys

for _p in ("/opt/trn_rl_repo",):
    if _p not in sys.path:
        sys.path.insert(0, _p)

import numpy as np
import ml_dtypes

import concourse.bass as bass
import concourse.bass_isa as bass_isa
import concourse.mybir as mybir
import concourse.tile as tile
from concourse import bass_utils

F32 = mybir.dt.float32
BF16 = mybir.dt.bfloat16
BFNP = ml_dtypes.bfloat16

# problem shapes (hardcoded per spec)
B, C, H, W = 64, 256, 21, 21
K, SHOT = 16, 5
L = H * W            # 441
N = SHOT * L         # 2205 support locations per class
NT = 18              # support row tiles after zero-padding
NP = NT * 128        # 2304 padded support rows
NCORES = 8
B_LOC = B // NCORES  # 8 queries per core
K_LOC = K // NCORES  # 2 classes per core

# stage-2 schedule knobs (tuned against TimelineSim)
S2_LAG = 10       # phase B trails phase A by this many pairs
S2_DVE_EVERY = 3  # every Nth pair drains via the DVE copy+mult chain
S2_FOLD_CYC = 3   # fold engine rotation period ...
S2_FOLD_DVE = 2   # ... of which this many go to DVE (rest GPSIMD)
S2_SIMCOPY_DVE = True   # sim copies on DVE instead of ACT
S2_DVE_PAT = (4, {1})   # optional (cycle, {hit indices}) for dve_pair
S2_SQBUFS = 14          # sq pool depth (must exceed S2_LAG + chain latency)
S2_RAWBUFS = 8          # raw pool depth for deferred DVE-pair mults
S1_WARM = 0             # stage-1 p-state warmup matmuls
S2_WARM = 0             # stage-2 p-state warmup matmuls
S2_MLAG = 1             # mult deferral distance cap (0 = inline)
S2_RED = 6              # k-rows per query summed on GPSIMD instead of PE
S2_MULT_GPS_ONLY = False  # all DVE-pair mults on GPSIMD

# fp8 / DoubleRow flags (DR needs the HW probe to pass; see module docstring)
S1_FP8 = True     # support tensor in fp8 e4m3 (halves stage-1 DMA)
S1_DR = True      # stage-1 gram via fp8 DoubleRow (2 row-tiles per matmul)
S2_DR = False     # stage-2 phase B via fp8 DoubleRow ones-matmul (fp8 squares)
F8 = mybir.dt.float8e4
F8NP = ml_dtypes.float8_e4m3
NT2 = NT // 2     # DR-paired row-tile count


# ---------------------------------------------------------------- waitfix
def _split_waits(nc):
    """This walrus build accepts at most ONE sync-wait command per
    instruction; hoist excess waits onto preceding NoOps (same engine)."""
    n_split = 0
    for fn in nc.m.functions:
        for blk in fn.blocks:
            new = []
            dirty = False
            for inst in blk.instructions:
                si = inst.sync_info
                waits = list(si.on_wait) if si is not None and si.on_wait else []
                if len(waits) > 1:
                    keep = waits[-1:]
                    for j, w in enumerate(waits[:-1]):
                        nop = mybir.InstNoOp(
                            name=f"{inst.name}-wsplit{j}", ins=[], outs=[]
                        )
                        nop.engine = inst.engine
                        nop.sync_info = mybir.SyncInfo(on_wait=[w], on_update=[])
                        new.append(nop)
                    inst.sync_info = mybir.SyncInfo(
                        on_wait=keep,
                        on_update=list(si.on_update) if si.on_update else [],
                    )
                    n_split += 1
                    dirty = True
                new.append(inst)
            if dirty:
                blk.instructions = new
    return n_split


# ---------------------------------------------------------------- stage 1
def build_stage1(split=True):
    """Per core: host-centered, zero-padded, pre-transposed support
    (K_LOC, NT2, 128, 2, C) [t2][p][u][c] (row = (2*t2+u)*128 + p) ->
    upper-triangular gram blocks packed as
      covT[k] = [ X^T[0:128] @ X  |  X^T[128:256] @ X[:,128:256] ]
    i.e. (K_LOC, 128, 384) f32. The interleaved [u] pairing keeps the DMA
    descriptor >= 512B in fp8 and is exactly the DoubleRow row pairing.
    The symmetric (1,0) block and the 1/(N-1) scale are applied on host.
    """
    dt_in = F8 if S1_FP8 else BF16
    nc = bass.Bass("TRN2", target_bir_lowering=False, debug=False)
    supt = nc.dram_tensor("support_t", [K_LOC, NT2, 128, 2, C], dt_in,
                          kind="ExternalInput").ap()
    covt = nc.dram_tensor("covt", [K_LOC, 128, 384], F32,
                          kind="ExternalOutput").ap()

    with tile.TileContext(nc) as tc:
        with (
            tc.tile_pool(name="xtpool", bufs=1) as xtpool,
            tc.tile_pool(name="covsb", bufs=2) as covsb,
            tc.tile_pool(name="cov_ps", bufs=2, space="PSUM") as cov_ps,
        ):
            xts = [xtpool.tile([128, NT2, 2, C], dt_in, name=f"xt{k}")
                   for k in range(K_LOC)]

            # DMA order chosen so class 0's PE loop is never starved and
            # class 1's data fully lands before class 0's matmuls finish.
            def load(k, t0, t1):
                nc.sync.dma_start(
                    xts[k][:, t0:t1, :, :],
                    supt[k, t0:t1].rearrange("t p u c -> p t (u c)"),
                )

            load(0, 0, 3)
            load(0, 3, NT2)
            load(1, 0, 5)
            load(1, 5, NT2)

            # p-state warmup on garbage SBUF into a scratch bank
            ccall = covsb.tile([128, K_LOC, 384], F32, tag="cc", name="ccall",
                               bufs=1)
            if S1_WARM:
                wsrc = covsb.tile([128, 512], BF16, tag="wsrc", name="wsrc",
                                  bufs=1)
                nc.gpsimd.memset(wsrc[:], 0.0)
                warm = cov_ps.tile([128, C], F32, tag="warm", name="warm")
                for w in range(S1_WARM):
                    nc.tensor.matmul(warm[:], wsrc[:, 0:128],
                                     wsrc[:, 256:512],
                                     start=(w == 0), stop=(w == S1_WARM - 1))

            for k in range(K_LOC):
                xt = xts[k]
                ps0 = cov_ps.tile([128, C], F32, tag="ps0", name=f"ps0_{k}")
                ps1 = cov_ps.tile([128, 128], F32, tag="ps1", name=f"ps1_{k}")
                # ps0 finishes first so its copy+DMA overlap the ps1 loop
                if S1_DR:
                    for t2 in range(NT2):
                        nc.tensor.matmul(
                            ps0[:], xt[:, t2, :, 0:128], xt[:, t2, :, :],
                            start=(t2 == 0), stop=(t2 == NT2 - 1),
                            perf_mode=mybir.MatmulPerfMode.DoubleRow,
                        )
                    for t2 in range(NT2):
                        nc.tensor.matmul(
                            ps1[:], xt[:, t2, :, 128:C], xt[:, t2, :, 128:C],
                            start=(t2 == 0), stop=(t2 == NT2 - 1),
                            perf_mode=mybir.MatmulPerfMode.DoubleRow,
                        )
                else:
                    for nt in range(NT):
                        t2, u = divmod(nt, 2)
                        nc.tensor.matmul(
                            ps0[:], xt[:, t2, u, 0:128], xt[:, t2, u, :],
                            start=(nt == 0), stop=(nt == NT - 1),
                        )
                    for nt in range(NT):
                        t2, u = divmod(nt, 2)
                        nc.tensor.matmul(
                            ps1[:], xt[:, t2, u, 128:C], xt[:, t2, u, 128:C],
                            start=(nt == 0), stop=(nt == NT - 1),
                        )
                # GPSIMD cannot access PSUM on HW: Act/DVE only
                nc.scalar.copy(ccall[:, k, 0:C], ps0[:])
                if k == K_LOC - 1:
                    # last class: fire the ps0 half while ps1 still computes
                    nc.sync.dma_start(covt[k, :, 0:C], ccall[:, k, 0:C])
                nc.vector.tensor_copy(ccall[:, k, C:384], ps1[:])
                if k == 0:
                    nc.sync.dma_start(covt[0], ccall[:, 0, :])
            nc.sync.dma_start(covt[K_LOC - 1, :, C:384],
                              ccall[:, K_LOC - 1, C:384])
    if split:
        _split_waits(nc)
    return nc


# ---------------------------------------------------------------- stage 2
def build_stage2(split=True):
    """Per core: q shard (B_LOC, 2, 128, L) bf16 + packed Cholesky blocks
    lch (K, 128, 3, 128) bf16 -> raw quadratic form out (B_LOC, K, L) f32
    (host applies 1/(||q||+eps)^2).

    lch slots: 0 = L[0:128, 0:128] (lower-tri), 1 = L[128:256, 0:128]
    (dense), 2 = L[128:256, 128:256] (lower-tri); slot L[0:128,128:256]
    is identically zero and skipped.
    """
    sq_dt = F8 if S2_DR else BF16
    nc = bass.Bass("TRN2", target_bir_lowering=False, debug=False)
    lch_in = nc.dram_tensor("lch", [K, 128, 3, 128], BF16,
                            kind="ExternalInput").ap()
    qm_in = nc.dram_tensor("qm", [B_LOC, 2, 128, L], BF16,
                           kind="ExternalInput").ap()
    if S2_DR:
        ones_in = nc.dram_tensor("onespk", [128, K, 2, K], F8,
                                 kind="ExternalInput").ap()
    else:
        ones_in = nc.dram_tensor("onespk", [128, K, K], BF16,
                                 kind="ExternalInput").ap()
    out = nc.dram_tensor("out", [B_LOC, K, L], F32, kind="ExternalOutput").ap()

    with tile.TileContext(nc) as tc:
        with (
            tc.tile_pool(name="singles", bufs=1) as singles,
            tc.tile_pool(name="sqp", bufs=S2_SQBUFS) as sqp,
            tc.tile_pool(name="rawp", bufs=S2_RAWBUFS) as rawp,
            tc.tile_pool(name="m1_ps", bufs=3, space="PSUM") as m1_ps,
            tc.tile_pool(name="sim_ps", bufs=2, space="PSUM") as sim_ps,
        ):
            qmm = singles.tile([128, 2, B_LOC, L], BF16)
            if S2_RED:
                redout = singles.tile([1, 2, S2_RED, L], F32, name="redout")
            lch = singles.tile([128, K, 3, 128], BF16)
            if S2_DR:
                onespk = singles.tile([128, K, 2, K], F8)
            else:
                onespk = singles.tile([128, K, K], BF16)
            outsb = singles.tile([16, B_LOC, L], F32)

            # onespk on the SWDGE (gpsimd) path so it doesn't occupy the
            # HWDGE while the big loads stream.
            nc.gpsimd.dma_start(onespk[:], ones_in)

            def dma_lch(k0, k1):
                nc.sync.dma_start(
                    lch[:, k0:k1], lch_in[k0:k1].rearrange("k p s d -> p k s d")
                )

            def dma_qm(b):
                # DMA aps allow at most 3 dims, so one dma_start per query
                nc.sync.dma_start(
                    qmm[:, :, b, :],
                    qm_in[b].rearrange("ct p l -> p ct l"),
                )

            # ordered by consumption: pair (b, k) needs lch[k] and qm[b]
            dma_lch(0, 1)
            nc.sync.dma_start(qmm[:, 0, 0, :], qm_in[0, 0])
            nc.sync.dma_start(qmm[:, 1, 0, :], qm_in[0, 1])
            dma_lch(1, 4)
            dma_qm(1)
            dma_lch(4, K)
            for b in range(2, B_LOC):
                dma_qm(b)

            # p-state warmup: matmuls on a dedicated scratch SBUF tile
            # (never DMA-written, so no dependency delays the loads) into
            # the first sim bank; the bank is later reset by the real
            # phase-B start=True, so values never escape.
            if S2_WARM:
                wsrc = singles.tile([128, L + 16], BF16, name="wsrc")
                nc.gpsimd.memset(wsrc[:], 0.0)
                warm = sim_ps.tile([128, L], F32, tag="simps", name="warm")
                for w in range(S2_WARM):
                    nc.tensor.matmul(warm[0:16, 0:L], wsrc[:, L:L + 16],
                                     wsrc[:, 0:L],
                                     start=(w == 0), stop=(w == S2_WARM - 1))

            # Only ACT can square straight from PSUM (one-PSUM-input rule
            # forbids a DVE self-mult there), and ACT's per-pair square time
            # exactly ties the PE, so every 4th pair drains via a DVE
            # copy+SBUF-self-mult chain instead and skips the fold (its
            # phase B uses two accumulating matmuls). Folds of ACT pairs
            # rotate over DVE/GPSIMD.
            LAG = S2_LAG  # phase B trails phase A by this many pairs

            def dve_pair(i):
                if S2_DVE_PAT is not None:
                    cyc, hits = S2_DVE_PAT
                    return (i % cyc) in hits
                return i % S2_DVE_EVERY == S2_DVE_EVERY - 1

            foldsel = [0]
            sqs = {}
            raws = {}
            simbank = {}

            def emit_A(i):
                b, k = divmod(i, K)
                if k == 0:
                    simbank[b] = sim_ps.tile([128, L], F32, tag="simps",
                                             name=f"sim_{b}")
                m = m1_ps.tile([128, 2, L], F32, tag="m1",
                               name=f"m_{b}_{k}", padded_shape=[128, 2, 512])
                nc.tensor.matmul(m[:, 0, :], lch[:, k, 0, :],
                                 qmm[:, 0, b, :], start=True, stop=False)
                nc.tensor.matmul(m[:, 0, :], lch[:, k, 1, :],
                                 qmm[:, 1, b, :], start=False, stop=True)
                nc.tensor.matmul(m[:, 1, :], lch[:, k, 2, :],
                                 qmm[:, 1, b, :], start=True, stop=True)
                nslot = 2 if S2_DR else 3
                sq = sqp.tile([128, nslot, L], sq_dt, tag="sq",
                              name=f"sq_{b}_{k}")
                red = k >= K - S2_RED  # phase B on GPSIMD for these
                if not dve_pair(i):
                    # merged 2-slab square (one ACT op, 882 cols)
                    nc.scalar.square(sq[:, 0:2, :], m[:, :, :])
                    if not S2_DR:
                        # near the tail, prefer the low-latency DVE fold
                        eng = (nc.vector
                               if (foldsel[0] % S2_FOLD_CYC < S2_FOLD_DVE
                                   or i >= B_LOC * K - 6)
                               else nc.gpsimd)
                        foldsel[0] += 1
                        eng.tensor_tensor(sq[:, 2, :], sq[:, 0, :],
                                          sq[:, 1, :], op=mybir.AluOpType.add)
                else:
                    # prompt PSUM drain (frees the m1 bank); the self-mult
                    # is deferred via emit_mult so it never delays the drain
                    raw = rawp.tile([128, 2, L], BF16, tag="raw",
                                    name=f"raw_{b}_{k}")
                    nc.vector.tensor_copy(raw[:], m[:, :, :])
                    raws[i] = raw
                sqs[i] = sq

            def emit_mult(i):
                # SBUF self-mult (2x mode when all operands are bf16);
                # in DR mode alternate DVE/GPSIMD to keep DVE under PE
                if S2_DR:
                    if S2_MULT_GPS_ONLY:
                        eng = nc.gpsimd
                    else:
                        eng = nc.gpsimd if foldsel[0] % 2 else nc.vector
                        foldsel[0] += 1
                else:
                    eng = nc.vector
                eng.tensor_tensor(sqs[i][:, 0:2, :], raws[i][:], raws[i][:],
                                  op=mybir.AluOpType.mult)
                b, k = divmod(i, K)
                if S2_RED and k >= K - S2_RED:
                    nc.vector.tensor_tensor(
                        sqs[i][:, 2, :],
                        sqs[i][:, 0, :], sqs[i][:, 1, :],
                        op=mybir.AluOpType.add)

            def emit_B(i):
                b, k = divmod(i, K)
                simps = simbank[b]
                klast = K - S2_RED - 1 if S2_RED else K - 1
                if S2_RED and k >= K - S2_RED:
                    # cross-partition sum on GPSIMD instead of a PE matmul
                    nc.gpsimd.tensor_reduce(
                        redout[0:1, b % 2, k - (K - S2_RED), :],
                        sqs[i][:, 2, :], axis=mybir.AxisListType.C,
                        op=mybir.AluOpType.add)
                    if k == K - 1:
                        nc.sync.dma_start(out[b, K - S2_RED:K, :],
                                          redout[0:1, b % 2])
                    return
                # one-hot column k of onespk routes the partition-sum of
                # sq into row k of the per-b sim bank; rows != k get +0.
                if S2_DR:
                    # fp8 DoubleRow contracts both 128-halves at once; the
                    # moving free dim caps at 512, so split L into 2 chunks.
                    for lo, hi in ((0, 221), (221, L)):
                        nc.tensor.matmul(
                            simps[0:16, lo:hi], onespk[:, k, :, :],
                            sqs[i][:, :, lo:hi],
                            start=(k == 0), stop=(k == K - 1),
                            perf_mode=mybir.MatmulPerfMode.DoubleRow,
                        )
                elif not dve_pair(i):
                    nc.tensor.matmul(simps[0:16, 0:L], onespk[:, k, :],
                                     sqs[i][:, 2, :],
                                     start=(k == 0), stop=(k == klast))
                else:
                    nc.tensor.matmul(simps[0:16, 0:L], onespk[:, k, :],
                                     sqs[i][:, 0, :],
                                     start=(k == 0), stop=False)
                    nc.tensor.matmul(simps[0:16, 0:L], onespk[:, k, :],
                                     sqs[i][:, 1, :],
                                     start=False, stop=(k == klast))
                if k == klast:
                    # sim copy is deferred into the next b's pipeline via
                    # emission position; it only waits on this b's last ones.
                    kb = K - S2_RED if S2_RED else K
                    if S2_SIMCOPY_DVE:
                        nc.vector.tensor_copy(outsb[0:kb, b, :],
                                              simbank[b][0:kb, 0:L])
                    else:
                        nc.scalar.copy(outsb[0:kb, b, :],
                                       simbank[b][0:kb, 0:L])
                    if b == 3:
                        nc.sync.dma_start(
                            out[0:4, 0:kb].rearrange("b k l -> k b l"),
                            outsb[0:kb, 0:4, :],
                        )
                    elif b >= 5:
                        b0 = 4 if b == 5 else b
                        nc.sync.dma_start(
                            out[b0:b + 1, 0:kb].rearrange("b k l -> k b l"),
                            outsb[0:kb, b0:b + 1, :],
                        )

            NPAIR = B_LOC * K
            MLAG = min(max(1, LAG - 3), S2_MLAG)
            for i in range(NPAIR):
                emit_A(i)
                if i >= MLAG and dve_pair(i - MLAG):
                    emit_mult(i - MLAG)
                if i >= LAG:
                    emit_B(i - LAG)
            for i in range(NPAIR - MLAG, NPAIR):
                if dve_pair(i):
                    emit_mult(i)
            for i in range(NPAIR - LAG, NPAIR):
                emit_B(i)

    if split:
        _split_waits(nc)
    return nc


# ---------------------------------------------------------------- host
_CACHE = {}


def _get(name):
    if name not in _CACHE:
        _CACHE[name] = build_stage1() if name == "s1" else build_stage2()
    return _CACHE[name]


LAST_RESULTS = {}


def kernel(query_features, support_features):
    q = np.asarray(query_features, dtype=np.float32).reshape(B, C, L)
    sup = np.asarray(support_features, dtype=np.float32).reshape(K, SHOT, C, L)

    # exact normalization folded into a final host-side scale
    n2 = np.einsum("bcl,bcl->bl", q, q, dtype=np.float64)
    inv = (1.0 / (np.sqrt(n2) + 1e-8) ** 2).astype(np.float32)

    # center support on host (exact f32/f64), pre-transpose + zero-pad,
    # then interleave 2 row-tiles per partition line: [k][t2][p][u][c]
    supf = np.ascontiguousarray(sup.transpose(0, 2, 1, 3)).reshape(K, C, N)
    supc = supf - supf.mean(axis=2, keepdims=True, dtype=np.float64).astype(
        np.float32)
    dt_in = F8NP if S1_FP8 else BFNP
    supp = np.zeros((K, NP, C), dtype=dt_in)
    supp[:, :N, :] = supc.transpose(0, 2, 1).astype(dt_in)
    supt = np.ascontiguousarray(
        supp.reshape(K, NT2, 2, 128, C).transpose(0, 1, 3, 2, 4)
    )

    nc1 = _get("s1")
    in1 = [{"support_t": supt[i * K_LOC:(i + 1) * K_LOC]}
           for i in range(NCORES)]
    r1 = bass_utils.run_bass_kernel_spmd(nc1, in1, core_ids=list(range(NCORES)))
    covT = np.concatenate([r["covt"] for r in r1.results], axis=0)
    cov0 = covT[:, :, 0:C]
    cov1 = covT[:, :, C:384]

    # reassemble symmetric covariance, scale, factor: sim = ||L^T q||^2
    covf = np.empty((K, C, C), dtype=np.float64)
    covf[:, :128, :] = cov0
    covf[:, 128:, 128:] = cov1
    covf[:, 128:, :128] = cov0[:, :, 128:].transpose(0, 2, 1)
    covf /= float(N - 1)
    Lch = np.linalg.cholesky(covf)

    # pack the three nonzero 128x128 blocks of L as lhsT[c_part, slot, d]
    lp = np.empty((K, 128, 3, 128), dtype=np.float32)
    lp[:, :, 0, :] = Lch[:, 0:128, 0:128]
    lp[:, :, 1, :] = Lch[:, 128:C, 0:128]
    lp[:, :, 2, :] = Lch[:, 128:C, 128:C]
    lpb = np.ascontiguousarray(lp.astype(BFNP))

    qmh = np.ascontiguousarray(q.astype(BFNP)).reshape(B, 2, 128, L)

    if S2_DR:
        onespk = np.zeros((128, K, 2, K), dtype=np.float32)
        onespk[:, np.arange(K), :, np.arange(K)] = 1.0
        onespk = onespk.astype(F8NP)
    else:
        onespk = np.zeros((128, K, K), dtype=np.float32)
        onespk[:, np.arange(K), np.arange(K)] = 1.0
        onespk = onespk.astype(BFNP)

    nc2 = _get("s2")
    in2 = [{"qm": qmh[i * B_LOC:(i + 1) * B_LOC], "lch": lpb,
            "onespk": onespk} for i in range(NCORES)]
    r2 = bass_utils.run_bass_kernel_spmd(nc2, in2, core_ids=list(range(NCORES)))
    outv = np.concatenate([r["out"] for r in r2.results], axis=0)

    LAST_RESULTS["s1"] = r1
    LAST_RESULTS["s2"] = r2
    res = outv * inv[:, None, :]
    return res.reshape(B, K * L).astype(np.float32)
